# revision 7
# baseline (speedup 1.0000x reference)
"""KAN layer on 8 Trainium2 NeuronCores (Bass/Tile).

Computes out = x @ base_weight.T + silu(x) @ spline_weight.sum(-1).T
for x:[8192,1024] f32, base_weight:[1024,1024] f32,
spline_weight:[1024,1024,8] f32 -> out:[8192,1024] f32.

Strategy (self-contained, hardcoded for these shapes):
  * 2D shard over the 8 cores: batch split R=2, out-features split C=4.
    Core (r, c) computes out[4096r:4096(r+1), 256c:256(c+1)].
  * Host prep is layout + dtype cast: x/spline ship bf16; the base
    path (x @ Wb^T) ships fp8e4 and runs DoubleRow matmuls (K=256 per
    MM) — it contributes only ~16% of the output variance, so fp8
    noise there is ~0.5% of the output. The spline path stays bf16.
  * Weight-stationary transposed formulation on device: psum[128o,
    512b] += W^T[128i, 128o].T @ x^T[128i, 512b], accumulating 4 fp8
    DoubleRow base MMs + 8 bf16 spline MMs per PSUM bank.
  * Two DMA queues: weights stream on the Scalar-engine HWDGE ring
    while x streams on the Sync ring; fp8 x chunks are interleaved
    ahead of the bf16 ones so base MMs can start early.
  * Staggered MM schedule: base-only groups for chunks 0-3 first,
    spline groups interleave once the DVE g-sum catches up. A short
    run of dummy warm-up matmuls right after the preamble brings the
    PE HAM clock to 2.4 GHz before real work arrives.
  * Output is written bf16 and upcast to f32 on gather.
    End-to-end rel err vs the f32 reference ~5e-3.
"""
import sys

for _p in ("/opt/trn_rl_repo",):
    if _p not in sys.path:
        sys.path.insert(0, _p)

import ml_dtypes
import numpy as np

import concourse.bass as bass  # noqa: F401  (bass must import before mybir use)
import concourse.mybir as mybir
import concourse.tile as tile
from concourse import bacc
from concourse.bass_utils import run_bass_kernel_spmd

P = 128
IN_F = 1024
G = 8
N_CORES = 8
R_SPLIT = 2
C_SPLIT = 4
B_LOC = 8192 // R_SPLIT      # 4096 batch rows per core
O_LOC = 1024 // C_SPLIT      # 256 out features per core
KT = IN_F // P               # 8 k-tiles over in_features
KB = KT // 2                 # 4 DoubleRow k-blocks of 256
M_CHUNK = 512
N_CHUNKS = B_LOC // M_CHUNK  # 8
OT = O_LOC // P              # 2 out-feature tiles of 128
N_WARM = 10                  # dummy MMs to warm the PE HAM clock

F32 = mybir.dt.float32
BF16 = mybir.dt.bfloat16
FP8 = mybir.dt.float8e4
AF = mybir.ActivationFunctionType
DR = mybir.MatmulPerfMode.DoubleRow
NP_BF16 = ml_dtypes.bfloat16
NP_FP8 = ml_dtypes.float8_e4m3

# Base-only MM groups run ahead; spline groups interleave after a lag.
SCHEDULE = [("B", 0), ("B", 1), ("B", 2), ("B", 3),
            ("S", 0), ("B", 4), ("S", 1), ("B", 5),
            ("S", 2), ("B", 6), ("S", 3), ("B", 7),
            ("S", 4), ("S", 5), ("S", 6), ("S", 7)]

_compiled = None


def _build_kernel():
    nc = bacc.Bacc(None, target_bir_lowering=False, num_devices=N_CORES)
    # x^T tiles: [ch, p, k, m] = x[r*4096 + ch*512 + m, k*128 + p]
    xt = nc.dram_tensor("xt", [N_CHUNKS, P, KT, M_CHUNK], BF16,
                        kind="ExternalInput")
    x8t = nc.dram_tensor("x8t", [N_CHUNKS, P, KT, M_CHUNK], FP8,
                         kind="ExternalInput")
    # W_base^T fp8 DoubleRow layout: bt8[p, kb, k2, o] =
    #   base_weight[c*256 + o, (2kb+k2)*128 + p]
    bt8 = nc.dram_tensor("bt8", [P, KB, 2, O_LOC], FP8, kind="ExternalInput")
    # spline^T: st[t, p, k2, g, o] = spline_weight[c*256+o, (2t+k2)*128+p, g]
    st = nc.dram_tensor("st", [KT // 2, P, 2, G, O_LOC], BF16,
                        kind="ExternalInput")
    # out^T tiles: out[ch, p, ot, m] = result[r*4096+ch*512+m, c*256+ot*128+p]
    out = nc.dram_tensor("out", [N_CHUNKS, P, OT, M_CHUNK], BF16,
                         kind="ExternalOutput")

    with tile.TileContext(nc) as tc:
        with (
            tc.tile_pool(name="wconst", bufs=1) as wconst,
            tc.tile_pool(name="sstage", bufs=4) as sstage,
            tc.tile_pool(name="xpool", bufs=N_CHUNKS) as xpool,
            tc.tile_pool(name="x8pool", bufs=4) as x8pool,
            tc.tile_pool(name="spool", bufs=N_CHUNKS) as spool,
            tc.tile_pool(name="psum", bufs=8, space="PSUM") as psum,
            tc.tile_pool(name="opool", bufs=3) as opool,
        ):
            # ---- PE warm-up: dummy MMs on a memset tile, no DMA deps ----
            wtile = wconst.tile([P, M_CHUNK], BF16, name="wtile")
            nc.vector.memset(wtile[:], 0.0)
            warm_pt = psum.tile([P, M_CHUNK], F32, name="pt", tag="pt")
            for _ in range(N_WARM):
                nc.tensor.matmul(warm_pt[:], wtile[:, 0:P], wtile[:],
                                 start=True, stop=True)

            # ---- weights on the (otherwise idle) GpSimd DMA ring ----
            wb8 = wconst.tile([P, KB, 2, O_LOC], FP8, name="wb8")
            nc.gpsimd.dma_start(wb8[:], bt8[:])
            stgs = []
            for t in range(KT // 2):
                stg = sstage.tile([P, 2, G, O_LOC], BF16, name="stg",
                                  tag="stg")
                nc.gpsimd.dma_start(stg[:], st[t])
                stgs.append(stg)

            # ---- x on the Sync DMA ring: all fp8 chunks land first ----
            x8s = []
            for ch in range(N_CHUNKS):
                x8 = x8pool.tile([P, KT, M_CHUNK], FP8, name="x8", tag="x8")
                nc.sync.dma_start(x8[:], x8t[ch])
                x8s.append(x8)
            xbs, sbs = [], []
            for ch in range(N_CHUNKS):
                xb = xpool.tile([P, KT, M_CHUNK], BF16, name="xb", tag="xb")
                nc.sync.dma_start(xb[:], xt[ch])
                sb = spool.tile([P, KT, M_CHUNK], BF16, name="sb", tag="sb")
                nc.scalar.activation(sb[:], xb[:], AF.Silu)
                xbs.append(xb)
                sbs.append(sb)

            # ---- spline g-sum on DVE (bf16, 2x mode) ----
            ws = wconst.tile([P, KT, O_LOC], BF16, name="ws")
            for t in range(KT // 2):
                stg = stgs[t]
                for k2 in range(2):
                    k = 2 * t + k2
                    a1 = sstage.tile([P, 4, O_LOC], BF16, name="a1", tag="a1")
                    nc.vector.tensor_add(a1[:], stg[:, k2, 0:4],
                                         stg[:, k2, 4:8])
                    a2 = sstage.tile([P, 2, O_LOC], BF16, name="a2", tag="a2")
                    nc.vector.tensor_add(a2[:], a1[:, 0:2], a1[:, 2:4])
                    nc.vector.tensor_add(ws[:, k], a2[:, 0], a2[:, 1])

            # ---- staggered MM schedule ----
            pts = {}
            for phase, ch in SCHEDULE:
                if phase == "B":
                    for o in range(OT):
                        pt = psum.tile([P, M_CHUNK], F32, name="pt", tag="pt")
                        pts[(ch, o)] = pt
                        osl = slice(P * o, P * (o + 1))
                        for kb in range(KB):
                            nc.tensor.matmul(
                                pt[:], wb8[:, kb, :, osl],
                                x8s[ch][:, 2 * kb:2 * kb + 2, :],
                                start=(kb == 0), stop=False,
                                perf_mode=DR,
                            )
                else:
                    otile = opool.tile([P, OT, M_CHUNK], BF16, name="otile",
                                       tag="otile")
                    for o in range(OT):
                        pt = pts.pop((ch, o))
                        osl = slice(P * o, P * (o + 1))
                        for k in range(KT):
                            nc.tensor.matmul(
                                pt[:], ws[:, k, osl], sbs[ch][:, k],
                                start=False, stop=(k == KT - 1),
                            )
                        nc.vector.tensor_copy(otile[:, o], pt[:])
                        nc.gpsimd.dma_start(out[ch, :, o], otile[:, o])
    nc.compile()
    return nc


def _get_compiled():
    global _compiled
    if _compiled is None:
        _compiled = _build_kernel()
    return _compiled


def _shard_inputs(x, base_weight, spline_weight):
    """Full f32 inputs -> 8 per-core in_maps (layout + dtype cast)."""
    x = np.asarray(x, dtype=np.float32)
    base_weight = np.asarray(base_weight, dtype=np.float32)
    spline_weight = np.asarray(spline_weight, dtype=np.float32)

    xt_T = np.ascontiguousarray(x.T)                        # [1024, 8192] f32
    btf = np.ascontiguousarray(base_weight.T)               # [1024 i, 1024 o]

    # Per row-group r: [ch, p, k, m] with one contiguous block per chunk
    xts, x8ts = [], []
    for r in range(R_SPLIT):
        xs = xt_T[:, B_LOC * r:B_LOC * (r + 1)]             # [1024, 4096]
        xs4 = (xs.reshape(KT, P, N_CHUNKS, M_CHUNK)
                 .transpose(2, 1, 0, 3))
        xts.append(np.ascontiguousarray(xs4.astype(NP_BF16)))
        x8ts.append(np.ascontiguousarray(xs4.astype(NP_FP8)))

    bts, sts = [], []
    for c in range(C_SPLIT):
        osl = slice(O_LOC * c, O_LOC * (c + 1))
        btc = (btf[:, osl]                         # [1024 i, 256 o]
               .reshape(KB, 2, P, O_LOC)           # [kb, k2, p, o]
               .transpose(2, 0, 1, 3)              # [p, kb, k2, o]
               .astype(NP_FP8))
        bts.append(np.ascontiguousarray(btc))
        stc = (spline_weight[osl]                  # [256 o, 1024 i, 8 g]
               .transpose(1, 2, 0)                 # [1024 i, 8 g, 256 o]
               .astype(NP_BF16)
               .reshape(KT // 2, 2, P, G, O_LOC)
               .transpose(0, 2, 1, 3, 4))          # [t, p, k2, g, o]
        sts.append(np.ascontiguousarray(stc))

    in_maps = []
    for core in range(N_CORES):
        r, c = divmod(core, C_SPLIT)
        in_maps.append({"xt": xts[r], "x8t": x8ts[r],
                        "bt8": bts[c], "st": sts[c]})
    return in_maps


def _gather_output(results):
    out = np.empty((8192, 1024), dtype=np.float32)
    for core in range(N_CORES):
        r, c = divmod(core, C_SPLIT)
        oc = results[core]["out"].astype(np.float32)  # [8 ch, 128 p, 2 ot, 512 m]
        oc = oc.transpose(0, 3, 2, 1).reshape(B_LOC, O_LOC)
        out[B_LOC * r:B_LOC * (r + 1), O_LOC * c:O_LOC * (c + 1)] = oc
    return out


def run(trace=False, **inputs):
    """Run on the 8 NeuronCores; returns (out, BassKernelResults)."""
    nc = _get_compiled()
    in_maps = _shard_inputs(**inputs)
    res = run_bass_kernel_spmd(
        nc, in_maps, core_ids=list(range(N_CORES)), trace=trace)
    return _gather_output(res.results), res


def kernel(**inputs) -> np.ndarray:
    out, _ = run(trace=False, **inputs)
    return out


# revision 8
# speedup vs baseline: 1.0401x; 1.0401x over previous
"""KAN layer on 8 Trainium2 NeuronCores (Bass/Tile).

Computes out = x @ base_weight.T + silu(x) @ spline_weight.sum(-1).T
for x:[8192,1024] f32, base_weight:[1024,1024] f32,
spline_weight:[1024,1024,8] f32 -> out:[8192,1024] f32.

Strategy (self-contained, hardcoded for these shapes):
  * 2D shard over the 8 cores: batch split R=2, out-features split C=4.
    Core (r, c) computes out[4096r:4096(r+1), 256c:256(c+1)].
  * Host prep is layout + dtype cast: x/spline ship bf16; the base
    path (x @ Wb^T) ships fp8e4 and runs DoubleRow matmuls (K=256 per
    MM) — it carries only ~16% of the output variance, so fp8 noise
    there is ~0.5% of the output. The spline path stays bf16.
  * Weight-stationary transposed formulation on device: psum[128o,
    512b] += W^T[128i, 128o].T @ x^T[128i, 512b].
  * Two decoupled passes over the batch: the base pass (fp8, fed by
    the small early-landing fp8 x stream) runs first and parks its
    PSUM tiles as bf16 partials in SBUF; the spline pass (bf16, fed
    by the later-landing spline weights + silu chain) runs second and
    folds the partials back in at PSUM-evict time. This matches the
    MM schedule to the order the data can arrive in.
  * DMA rings: x on Sync (fp8 chunks prioritized), weights on Scalar,
    outputs on GpSimd. The spline g-axis reduce runs on the otherwise
    idle GpSimd engine; PSUM evicts/combines on Vector; silu on
    Scalar. A short run of dummy warm-up matmuls right after the
    preamble brings the PE HAM clock to 2.4 GHz before real work.
  * Output is written bf16 and upcast to f32 on gather.
    End-to-end rel err vs the f32 reference ~8e-3.
"""
import sys

for _p in ("/opt/trn_rl_repo",):
    if _p not in sys.path:
        sys.path.insert(0, _p)

import ml_dtypes
import numpy as np

import concourse.bass as bass  # noqa: F401  (bass must import before mybir use)
import concourse.mybir as mybir
import concourse.tile as tile
from concourse import bacc
from concourse.bass_utils import run_bass_kernel_spmd

P = 128
IN_F = 1024
G = 8
N_CORES = 8
R_SPLIT = 2
C_SPLIT = 4
B_LOC = 8192 // R_SPLIT      # 4096 batch rows per core
O_LOC = 1024 // C_SPLIT      # 256 out features per core
KT = IN_F // P               # 8 k-tiles over in_features
KB = KT // 2                 # 4 DoubleRow k-blocks of 256
M_CHUNK = 512
N_CHUNKS = B_LOC // M_CHUNK  # 8
OT = O_LOC // P              # 2 out-feature tiles of 128
N_WARM = 10                  # dummy MMs to warm the PE HAM clock

F32 = mybir.dt.float32
BF16 = mybir.dt.bfloat16
FP8 = mybir.dt.float8e4
AF = mybir.ActivationFunctionType
DR = mybir.MatmulPerfMode.DoubleRow
NP_BF16 = ml_dtypes.bfloat16
NP_FP8 = ml_dtypes.float8_e4m3

# Sync-ring DMA priority order: which x transfer lands when.
# fp8 chunks (fuel for the early base pass) go first, with the first
# bf16 chunks (fuel for the silu chain) interleaved between them.
X_ORDER = [("x8", 0), ("x8", 1), ("xb", 0), ("x8", 2), ("x8", 3),
           ("xb", 1), ("x8", 4), ("x8", 5), ("x8", 6), ("x8", 7),
           ("xb", 2), ("xb", 3), ("xb", 4), ("xb", 5), ("xb", 6),
           ("xb", 7)]

_compiled = None


def _build_kernel():
    nc = bacc.Bacc(None, target_bir_lowering=False, num_devices=N_CORES)
    # x^T tiles: [ch, p, k, m] = x[r*4096 + ch*512 + m, k*128 + p]
    xt = nc.dram_tensor("xt", [N_CHUNKS, P, KT, M_CHUNK], BF16,
                        kind="ExternalInput")
    x8t = nc.dram_tensor("x8t", [N_CHUNKS, P, KT, M_CHUNK], FP8,
                         kind="ExternalInput")
    # W_base^T fp8 DoubleRow layout: bt8[p, kb, k2, o] =
    #   base_weight[c*256 + o, (2kb+k2)*128 + p]
    bt8 = nc.dram_tensor("bt8", [P, KB, 2, O_LOC], FP8, kind="ExternalInput")
    # spline^T: st[t, p, k2, g, o] = spline_weight[c*256+o, (2t+k2)*128+p, g]
    st = nc.dram_tensor("st", [KT // 2, P, 2, G, O_LOC], BF16,
                        kind="ExternalInput")
    # out^T tiles: out[ch, p, ot, m] = result[r*4096+ch*512+m, c*256+ot*128+p]
    out = nc.dram_tensor("out", [N_CHUNKS, P, OT, M_CHUNK], BF16,
                         kind="ExternalOutput")

    with tile.TileContext(nc) as tc:
        with (
            tc.tile_pool(name="wconst", bufs=1) as wconst,
            tc.tile_pool(name="sstage", bufs=4) as sstage,
            tc.tile_pool(name="gpool", bufs=2) as gpool,
            tc.tile_pool(name="x8pool", bufs=N_CHUNKS) as x8pool,
            tc.tile_pool(name="xpool", bufs=5) as xpool,
            tc.tile_pool(name="spool", bufs=N_CHUNKS) as spool,
            tc.tile_pool(name="ppool", bufs=N_CHUNKS) as ppool,
            tc.tile_pool(name="psum", bufs=8, space="PSUM") as psum,
            tc.tile_pool(name="opool", bufs=3) as opool,
        ):
            # ---- PE warm-up: dummy MMs on a memset tile, no DMA deps ----
            wtile = wconst.tile([P, M_CHUNK], BF16, name="wtile")
            nc.vector.memset(wtile[:], 0.0)
            warm_pt = psum.tile([P, M_CHUNK], F32, name="pt", tag="pt")
            for _ in range(N_WARM):
                nc.tensor.matmul(warm_pt[:], wtile[:, 0:P], wtile[:],
                                 start=True, stop=True)

            # ---- weights on the Scalar-engine DMA ring ----
            wb8 = wconst.tile([P, KB, 2, O_LOC], FP8, name="wb8")
            nc.scalar.dma_start(wb8[:], bt8[:])
            stgs = []
            for t in range(KT // 2):
                stg = sstage.tile([P, 2, G, O_LOC], BF16, name="stg",
                                  tag="stg")
                nc.scalar.dma_start(stg[:], st[t])
                stgs.append(stg)

            # ---- x on the Sync ring in explicit priority order ----
            x8s = [None] * N_CHUNKS
            xbs = [None] * N_CHUNKS
            sbs = [None] * N_CHUNKS
            for kind, ch in X_ORDER:
                if kind == "x8":
                    x8 = x8pool.tile([P, KT, M_CHUNK], FP8, name="x8",
                                     tag="x8")
                    nc.sync.dma_start(x8[:], x8t[ch])
                    x8s[ch] = x8
                else:
                    xb = xpool.tile([P, KT, M_CHUNK], BF16, name="xb",
                                    tag="xb")
                    nc.sync.dma_start(xb[:], xt[ch])
                    sb = spool.tile([P, KT, M_CHUNK], BF16, name="sb",
                                    tag="sb")
                    nc.scalar.activation(sb[:], xb[:], AF.Silu)
                    xbs[ch] = xb
                    sbs[ch] = sb

            # ---- spline g-sum on the (otherwise idle) GpSimd engine ----
            ws = wconst.tile([P, KT, O_LOC], BF16, name="ws")
            for t in range(KT // 2):
                stg = stgs[t]
                for k2 in range(2):
                    k = 2 * t + k2
                    a1 = gpool.tile([P, 4, O_LOC], BF16, name="a1", tag="a1")
                    nc.gpsimd.tensor_add(a1[:], stg[:, k2, 0:4],
                                         stg[:, k2, 4:8])
                    a2 = gpool.tile([P, 2, O_LOC], BF16, name="a2", tag="a2")
                    nc.gpsimd.tensor_add(a2[:], a1[:, 0:2], a1[:, 2:4])
                    nc.gpsimd.tensor_add(ws[:, k], a2[:, 0], a2[:, 1])

            # ---- pass 1: fp8 DoubleRow base matmuls -> bf16 partials ----
            parts = []
            for ch in range(N_CHUNKS):
                part = ppool.tile([P, OT, M_CHUNK], BF16, name="part",
                                  tag="part")
                for o in range(OT):
                    pt = psum.tile([P, M_CHUNK], F32, name="pt", tag="pt")
                    osl = slice(P * o, P * (o + 1))
                    for kb in range(KB):
                        nc.tensor.matmul(
                            pt[:], wb8[:, kb, :, osl],
                            x8s[ch][:, 2 * kb:2 * kb + 2, :],
                            start=(kb == 0), stop=(kb == KB - 1),
                            perf_mode=DR,
                        )
                    nc.vector.tensor_copy(part[:, o], pt[:])
                parts.append(part)

            # ---- pass 2: bf16 spline matmuls, fold partials at evict ----
            for ch in range(N_CHUNKS):
                otile = opool.tile([P, OT, M_CHUNK], BF16, name="otile",
                                   tag="otile")
                for o in range(OT):
                    pt = psum.tile([P, M_CHUNK], F32, name="pt", tag="pt")
                    osl = slice(P * o, P * (o + 1))
                    for k in range(KT):
                        nc.tensor.matmul(
                            pt[:], ws[:, k, osl], sbs[ch][:, k],
                            start=(k == 0), stop=(k == KT - 1),
                        )
                    nc.vector.tensor_add(otile[:, o], pt[:], parts[ch][:, o])
                    nc.gpsimd.dma_start(out[ch, :, o], otile[:, o])
    nc.compile()
    return nc


def _get_compiled():
    global _compiled
    if _compiled is None:
        _compiled = _build_kernel()
    return _compiled


def _shard_inputs(x, base_weight, spline_weight):
    """Full f32 inputs -> 8 per-core in_maps (layout + dtype cast)."""
    x = np.asarray(x, dtype=np.float32)
    base_weight = np.asarray(base_weight, dtype=np.float32)
    spline_weight = np.asarray(spline_weight, dtype=np.float32)

    xt_T = np.ascontiguousarray(x.T)                        # [1024, 8192] f32
    btf = np.ascontiguousarray(base_weight.T)               # [1024 i, 1024 o]

    # Per row-group r: [ch, p, k, m] with one contiguous block per chunk
    xts, x8ts = [], []
    for r in range(R_SPLIT):
        xs = xt_T[:, B_LOC * r:B_LOC * (r + 1)]             # [1024, 4096]
        xs4 = (xs.reshape(KT, P, N_CHUNKS, M_CHUNK)
                 .transpose(2, 1, 0, 3))
        xts.append(np.ascontiguousarray(xs4.astype(NP_BF16)))
        x8ts.append(np.ascontiguousarray(xs4.astype(NP_FP8)))

    bts, sts = [], []
    for c in range(C_SPLIT):
        osl = slice(O_LOC * c, O_LOC * (c + 1))
        btc = (btf[:, osl]                         # [1024 i, 256 o]
               .reshape(KB, 2, P, O_LOC)           # [kb, k2, p, o]
               .transpose(2, 0, 1, 3)              # [p, kb, k2, o]
               .astype(NP_FP8))
        bts.append(np.ascontiguousarray(btc))
        stc = (spline_weight[osl]                  # [256 o, 1024 i, 8 g]
               .transpose(1, 2, 0)                 # [1024 i, 8 g, 256 o]
               .astype(NP_BF16)
               .reshape(KT // 2, 2, P, G, O_LOC)
               .transpose(0, 2, 1, 3, 4))          # [t, p, k2, g, o]
        sts.append(np.ascontiguousarray(stc))

    in_maps = []
    for core in range(N_CORES):
        r, c = divmod(core, C_SPLIT)
        in_maps.append({"xt": xts[r], "x8t": x8ts[r],
                        "bt8": bts[c], "st": sts[c]})
    return in_maps


def _gather_output(results):
    out = np.empty((8192, 1024), dtype=np.float32)
    for core in range(N_CORES):
        r, c = divmod(core, C_SPLIT)
        oc = results[core]["out"].astype(np.float32)  # [8 ch, 128 p, 2 ot, 512 m]
        oc = oc.transpose(0, 3, 2, 1).reshape(B_LOC, O_LOC)
        out[B_LOC * r:B_LOC * (r + 1), O_LOC * c:O_LOC * (c + 1)] = oc
    return out


def run(trace=False, **inputs):
    """Run on the 8 NeuronCores; returns (out, BassKernelResults)."""
    nc = _get_compiled()
    in_maps = _shard_inputs(**inputs)
    res = run_bass_kernel_spmd(
        nc, in_maps, core_ids=list(range(N_CORES)), trace=trace)
    return _gather_output(res.results), res


def kernel(**inputs) -> np.ndarray:
    out, _ = run(trace=False, **inputs)
    return out


# revision 11
# speedup vs baseline: 1.1726x; 1.1274x over previous
"""KAN layer on 8 Trainium2 NeuronCores (Bass/Tile).

Computes out = x @ base_weight.T + silu(x) @ spline_weight.sum(-1).T
for x:[8192,1024] f32, base_weight:[1024,1024] f32,
spline_weight:[1024,1024,8] f32 -> out:[8192,1024] f32.

Strategy (self-contained, hardcoded for these shapes):
  * 2D shard over the 8 cores: batch split R=2, out-features split C=4.
    Core (r, c) computes out[4096r:4096(r+1), 256c:256(c+1)].
  * Host prep is layout + dtype cast: x/spline ship bf16; the base
    path (x @ Wb^T) ships fp8e4 and runs DoubleRow matmuls (K=256 per
    MM) — it carries only ~16% of the output variance, so fp8 noise
    there is ~0.5% of the output. The spline path stays bf16.
  * Weight-stationary transposed formulation on device: psum[128o,
    512b] += W^T[128i, 128o].T @ x^T[128i, 512b].
  * Two decoupled passes over the batch: the base pass (fp8, fed by
    the small early-landing fp8 x stream) runs first and parks its
    PSUM tiles as bf16 partials in SBUF; the spline pass (bf16, fed
    by the later-landing spline weights + silu chain) runs second and
    folds the partials back in at PSUM-evict time. This matches the
    MM schedule to the order the data can arrive in.
  * DMA rings: x on Sync (fp8 chunks prioritized), weights on Scalar,
    outputs on GpSimd. The spline g-axis reduce runs on the otherwise
    idle GpSimd engine; PSUM evicts/combines on Vector; silu on
    Scalar. A short run of dummy warm-up matmuls right after the
    preamble brings the PE HAM clock to 2.4 GHz before real work.
  * Output is written bf16 and upcast to f32 on gather.
    End-to-end rel err vs the f32 reference ~8e-3.
"""
import sys

for _p in ("/opt/trn_rl_repo",):
    if _p not in sys.path:
        sys.path.insert(0, _p)

import ml_dtypes
import numpy as np

import concourse.bass as bass  # noqa: F401  (bass must import before mybir use)
import concourse.mybir as mybir
import concourse.tile as tile
from concourse import bacc
from concourse.bass_utils import run_bass_kernel_spmd

P = 128
IN_F = 1024
G = 8
N_CORES = 8
R_SPLIT = 2
C_SPLIT = 4
B_LOC = 8192 // R_SPLIT      # 4096 batch rows per core
O_LOC = 1024 // C_SPLIT      # 256 out features per core
KT = IN_F // P               # 8 k-tiles over in_features
KB = KT // 2                 # 4 DoubleRow k-blocks of 256
M_CHUNK = 512
N_CHUNKS = B_LOC // M_CHUNK  # 8
OT = O_LOC // P              # 2 out-feature tiles of 128
N_WARM = 10                  # dummy MMs to warm the PE HAM clock

F32 = mybir.dt.float32
BF16 = mybir.dt.bfloat16
FP8 = mybir.dt.float8e4
AF = mybir.ActivationFunctionType
DR = mybir.MatmulPerfMode.DoubleRow
NP_BF16 = ml_dtypes.bfloat16
NP_FP8 = ml_dtypes.float8_e4m3

# Sync-ring DMA priority order: which x transfer lands when.
# fp8 chunks (fuel for the early base pass) go first, with the first
# bf16 chunks (fuel for the silu chain) interleaved between them.
X_ORDER = [("x8", 0), ("x8", 1), ("x8", 2), ("x8", 3), ("x8", 4),
           ("xb", 0), ("x8", 5), ("x8", 6), ("xb", 1), ("x8", 7),
           ("xb", 2), ("xb", 3), ("xb", 4), ("xb", 5), ("xb", 6),
           ("xb", 7)]

_compiled = None


def _build_kernel():
    nc = bacc.Bacc(None, target_bir_lowering=False, num_devices=N_CORES)
    # x^T tiles: [ch, p, k, m] = x[r*4096 + ch*512 + m, k*128 + p]
    xt = nc.dram_tensor("xt", [N_CHUNKS, P, KT, M_CHUNK], BF16,
                        kind="ExternalInput")
    x8t = nc.dram_tensor("x8t", [N_CHUNKS, P, KT, M_CHUNK], FP8,
                         kind="ExternalInput")
    # W_base^T fp8 DoubleRow layout: bt8[p, kb, k2, o] =
    #   base_weight[c*256 + o, (2kb+k2)*128 + p]
    bt8 = nc.dram_tensor("bt8", [P, KB, 2, O_LOC], FP8, kind="ExternalInput")
    # spline^T: st[t, p, k2, g, o] = spline_weight[c*256+o, (2t+k2)*128+p, g]
    st = nc.dram_tensor("st", [KT // 2, P, 2, G, O_LOC], BF16,
                        kind="ExternalInput")
    # out^T tiles: out[ch, p, ot, m] = result[r*4096+ch*512+m, c*256+ot*128+p]
    out = nc.dram_tensor("out", [N_CHUNKS, P, OT, M_CHUNK], BF16,
                         kind="ExternalOutput")

    with tile.TileContext(nc) as tc:
        with (
            tc.tile_pool(name="wconst", bufs=1) as wconst,
            tc.tile_pool(name="sstage", bufs=4) as sstage,
            tc.tile_pool(name="gpool", bufs=2) as gpool,
            tc.tile_pool(name="x8pool", bufs=N_CHUNKS) as x8pool,
            tc.tile_pool(name="xpool", bufs=5) as xpool,
            tc.tile_pool(name="spool", bufs=N_CHUNKS) as spool,
            tc.tile_pool(name="ppool", bufs=N_CHUNKS) as ppool,
            tc.tile_pool(name="psum", bufs=8, space="PSUM") as psum,
            tc.tile_pool(name="opool", bufs=3) as opool,
        ):
            # ---- PE warm-up: dummy MMs on a memset tile, no DMA deps ----
            wtile = wconst.tile([P, M_CHUNK], BF16, name="wtile")
            nc.vector.memset(wtile[:], 0.0)
            # Dummy silu primes the ACT table load off the critical path.
            sdum = wconst.tile([P, 4], BF16, name="sdum")
            nc.scalar.activation(sdum[:], wtile[:, 0:4], AF.Silu)
            warm_pt = psum.tile([P, M_CHUNK], F32, name="pt", tag="pt")
            for _ in range(N_WARM):
                nc.tensor.matmul(warm_pt[:], wtile[:, 0:P], wtile[:],
                                 start=True, stop=True)

            # ---- weights on the Scalar-engine DMA ring ----
            wb8 = wconst.tile([P, KB, 2, O_LOC], FP8, name="wb8")
            nc.scalar.dma_start(wb8[:], bt8[:])
            stgs = []
            for t in range(KT // 2):
                stg = sstage.tile([P, 2, G, O_LOC], BF16, name="stg",
                                  tag="stg")
                nc.scalar.dma_start(stg[:], st[t])
                stgs.append(stg)

            # ---- x on the Sync ring in explicit priority order ----
            x8s = [None] * N_CHUNKS
            xbs = [None] * N_CHUNKS
            sbs = [None] * N_CHUNKS
            for kind, ch in X_ORDER:
                if kind == "x8":
                    x8 = x8pool.tile([P, KT, M_CHUNK], FP8, name="x8",
                                     tag="x8")
                    nc.sync.dma_start(x8[:], x8t[ch])
                    x8s[ch] = x8
                else:
                    xb = xpool.tile([P, KT, M_CHUNK], BF16, name="xb",
                                    tag="xb")
                    nc.sync.dma_start(xb[:], xt[ch])
                    sb = spool.tile([P, KT, M_CHUNK], BF16, name="sb",
                                    tag="sb")
                    nc.scalar.activation(sb[:], xb[:], AF.Silu)
                    xbs[ch] = xb
                    sbs[ch] = sb

            # ---- spline g-sum on DVE, interleaved into pass 1 below ----
            ws = wconst.tile([P, KT, O_LOC], BF16, name="ws")

            def gsum(k):
                stg = stgs[k // 2]
                k2 = k % 2
                a1 = gpool.tile([P, 4, O_LOC], BF16, name="a1", tag="a1")
                nc.vector.tensor_add(a1[:], stg[:, k2, 0:4], stg[:, k2, 4:8])
                a2 = gpool.tile([P, 2, O_LOC], BF16, name="a2", tag="a2")
                nc.vector.tensor_add(a2[:], a1[:, 0:2], a1[:, 2:4])
                nc.vector.tensor_add(ws[:, k], a2[:, 0], a2[:, 1])

            # ---- pass 1: fp8 DoubleRow base matmuls -> bf16 partials ----
            # One g-sum k-tile is threaded between each chunk's evicts so
            # the DVE never blocks the PSUM rotation waiting on spline DMA.
            parts = []
            for ch in range(N_CHUNKS):
                part = ppool.tile([P, OT, M_CHUNK], BF16, name="part",
                                  tag="part")
                for o in range(OT):
                    pt = psum.tile([P, M_CHUNK], F32, name="pt", tag="pt")
                    osl = slice(P * o, P * (o + 1))
                    for kb in range(KB):
                        nc.tensor.matmul(
                            pt[:], wb8[:, kb, :, osl],
                            x8s[ch][:, 2 * kb:2 * kb + 2, :],
                            start=(kb == 0), stop=(kb == KB - 1),
                            perf_mode=DR,
                        )
                    nc.vector.tensor_copy(part[:, o], pt[:])
                parts.append(part)
                gsum(ch)

            # ---- pass 2: bf16 spline matmuls, fold partials at evict ----
            for ch in range(N_CHUNKS):
                otile = opool.tile([P, OT, M_CHUNK], BF16, name="otile",
                                   tag="otile")
                for o in range(OT):
                    pt = psum.tile([P, M_CHUNK], F32, name="pt", tag="pt")
                    osl = slice(P * o, P * (o + 1))
                    for k in range(KT):
                        nc.tensor.matmul(
                            pt[:], ws[:, k, osl], sbs[ch][:, k],
                            start=(k == 0), stop=(k == KT - 1),
                        )
                    nc.vector.tensor_add(otile[:, o], pt[:], parts[ch][:, o])
                    nc.gpsimd.dma_start(out[ch, :, o], otile[:, o])
    nc.compile()
    return nc


def _get_compiled():
    global _compiled
    if _compiled is None:
        _compiled = _build_kernel()
    return _compiled


def _shard_inputs(x, base_weight, spline_weight):
    """Full f32 inputs -> 8 per-core in_maps (layout + dtype cast)."""
    x = np.asarray(x, dtype=np.float32)
    base_weight = np.asarray(base_weight, dtype=np.float32)
    spline_weight = np.asarray(spline_weight, dtype=np.float32)

    xt_T = np.ascontiguousarray(x.T)                        # [1024, 8192] f32
    btf = np.ascontiguousarray(base_weight.T)               # [1024 i, 1024 o]

    # Per row-group r: [ch, p, k, m] with one contiguous block per chunk
    xts, x8ts = [], []
    for r in range(R_SPLIT):
        xs = xt_T[:, B_LOC * r:B_LOC * (r + 1)]             # [1024, 4096]
        xs4 = (xs.reshape(KT, P, N_CHUNKS, M_CHUNK)
                 .transpose(2, 1, 0, 3))
        xts.append(np.ascontiguousarray(xs4.astype(NP_BF16)))
        x8ts.append(np.ascontiguousarray(xs4.astype(NP_FP8)))

    bts, sts = [], []
    for c in range(C_SPLIT):
        osl = slice(O_LOC * c, O_LOC * (c + 1))
        btc = (btf[:, osl]                         # [1024 i, 256 o]
               .reshape(KB, 2, P, O_LOC)           # [kb, k2, p, o]
               .transpose(2, 0, 1, 3)              # [p, kb, k2, o]
               .astype(NP_FP8))
        bts.append(np.ascontiguousarray(btc))
        stc = (spline_weight[osl]                  # [256 o, 1024 i, 8 g]
               .transpose(1, 2, 0)                 # [1024 i, 8 g, 256 o]
               .astype(NP_BF16)
               .reshape(KT // 2, 2, P, G, O_LOC)
               .transpose(0, 2, 1, 3, 4))          # [t, p, k2, g, o]
        sts.append(np.ascontiguousarray(stc))

    in_maps = []
    for core in range(N_CORES):
        r, c = divmod(core, C_SPLIT)
        in_maps.append({"xt": xts[r], "x8t": x8ts[r],
                        "bt8": bts[c], "st": sts[c]})
    return in_maps


def _gather_output(results):
    out = np.empty((8192, 1024), dtype=np.float32)
    for core in range(N_CORES):
        r, c = divmod(core, C_SPLIT)
        oc = results[core]["out"].astype(np.float32)  # [8 ch, 128 p, 2 ot, 512 m]
        oc = oc.transpose(0, 3, 2, 1).reshape(B_LOC, O_LOC)
        out[B_LOC * r:B_LOC * (r + 1), O_LOC * c:O_LOC * (c + 1)] = oc
    return out


def run(trace=False, **inputs):
    """Run on the 8 NeuronCores; returns (out, BassKernelResults)."""
    nc = _get_compiled()
    in_maps = _shard_inputs(**inputs)
    res = run_bass_kernel_spmd(
        nc, in_maps, core_ids=list(range(N_CORES)), trace=trace)
    return _gather_output(res.results), res


def kernel(**inputs) -> np.ndarray:
    out, _ = run(trace=False, **inputs)
    return out


# revision 13
# speedup vs baseline: 1.1900x; 1.0148x over previous
"""KAN layer on 8 Trainium2 NeuronCores (Bass/Tile).

Computes out = x @ base_weight.T + silu(x) @ spline_weight.sum(-1).T
for x:[8192,1024] f32, base_weight:[1024,1024] f32,
spline_weight:[1024,1024,8] f32 -> out:[8192,1024] f32.

Strategy (self-contained, hardcoded for these shapes):
  * 2D shard over the 8 cores: batch split R=2, out-features split C=4.
    Core (r, c) computes out[4096r:4096(r+1), 256c:256(c+1)].
  * Host prep is layout + dtype cast: x/spline ship bf16; the base
    path (x @ Wb^T) ships fp8e4 and runs DoubleRow matmuls (K=256 per
    MM) — it carries only ~16% of the output variance, so fp8 noise
    there is ~0.5% of the output. The spline path stays bf16.
  * Weight-stationary transposed formulation on device: psum[128o,
    512b] += W^T[128i, 128o].T @ x^T[128i, 512b].
  * Two decoupled passes over the batch: the base pass (fp8, fed by
    the small fp8 x stream that lands first) runs early and parks its
    PSUM tiles as bf16 partials in SBUF; the spline pass (bf16, fed
    by the later-landing spline weights + serial silu chain) runs
    second and folds the partials back in at PSUM-evict time.
  * ALL inputs ride ONE hand-ordered Sync-ring FIFO (SDMA round-robin
    is per-packet, so competing rings dilute each other's bandwidth —
    a single ring gets the full ~430 GB/s in exactly the order the
    compute needs). Outputs ride the otherwise idle GpSimd ring.
  * Spline g-axis reduce on Vector, positioned in the queue to match
    spline-weight arrival; silu on Scalar (primed by a dummy op so
    its table load is off the critical path); PE warm-up MMs bring
    the HAM clock to 2.4 GHz before real work.
  * Output is written bf16 and upcast to f32 on gather.
    End-to-end rel err vs the f32 reference ~8e-3.
"""
import sys

for _p in ("/opt/trn_rl_repo",):
    if _p not in sys.path:
        sys.path.insert(0, _p)

import ml_dtypes
import numpy as np

import concourse.bass as bass  # noqa: F401  (bass must import before mybir use)
import concourse.mybir as mybir
import concourse.tile as tile
from concourse import bacc
from concourse.bass_utils import run_bass_kernel_spmd

P = 128
IN_F = 1024
G = 8
N_CORES = 8
R_SPLIT = 2
C_SPLIT = 4
B_LOC = 8192 // R_SPLIT      # 4096 batch rows per core
O_LOC = 1024 // C_SPLIT      # 256 out features per core
KT = IN_F // P               # 8 k-tiles over in_features
KB = KT // 2                 # 4 DoubleRow k-blocks of 256
M_CHUNK = 512
N_CHUNKS = B_LOC // M_CHUNK  # 8
OT = O_LOC // P              # 2 out-feature tiles of 128
N_WARM = 8                   # dummy MMs to warm the PE HAM clock

F32 = mybir.dt.float32
BF16 = mybir.dt.bfloat16
FP8 = mybir.dt.float8e4
AF = mybir.ActivationFunctionType
DR = mybir.MatmulPerfMode.DoubleRow
NP_BF16 = ml_dtypes.bfloat16
NP_FP8 = ml_dtypes.float8_e4m3

# Single-ring input order: fp8 x fuels the early base pass, the first
# bf16 chunks keep the silu chain fed, spline weights land in time for
# the spline pass, remaining bf16 chunks trail.
IN_ORDER = [("x8", 0), ("x8", 1), ("xb", 0), ("x8", 2), ("x8", 3),
            ("st", 0), ("x8", 4), ("x8", 5), ("st", 1), ("xb", 1),
            ("x8", 6), ("x8", 7), ("st", 2), ("st", 3), ("xb", 2),
            ("xb", 3), ("xb", 4), ("xb", 5), ("xb", 6), ("xb", 7)]

# g-sum k-tiles threaded into the DVE queue after these base-pass
# chunks' evicts (matched to spline-weight arrival times).
GSUM_AFTER = {3: [0], 4: [1], 5: [2], 6: [3], 7: [4, 5, 6, 7]}

_compiled = None


def _build_kernel():
    nc = bacc.Bacc(None, target_bir_lowering=False, num_devices=N_CORES)
    # x^T tiles: [ch, p, k, m] = x[r*4096 + ch*512 + m, k*128 + p]
    xt = nc.dram_tensor("xt", [N_CHUNKS, P, KT, M_CHUNK], BF16,
                        kind="ExternalInput")
    x8t = nc.dram_tensor("x8t", [N_CHUNKS, P, KT, M_CHUNK], FP8,
                         kind="ExternalInput")
    # W_base^T fp8 DoubleRow layout: bt8[p, kb, k2, o] =
    #   base_weight[c*256 + o, (2kb+k2)*128 + p]
    bt8 = nc.dram_tensor("bt8", [P, KB, 2, O_LOC], FP8, kind="ExternalInput")
    # spline^T: st[t, p, k2, g, o] = spline_weight[c*256+o, (2t+k2)*128+p, g]
    st = nc.dram_tensor("st", [KT // 2, P, 2, G, O_LOC], BF16,
                        kind="ExternalInput")
    # out^T tiles: out[ch, p, ot, m] = result[r*4096+ch*512+m, c*256+ot*128+p]
    out = nc.dram_tensor("out", [N_CHUNKS, P, OT, M_CHUNK], BF16,
                         kind="ExternalOutput")

    with tile.TileContext(nc) as tc:
        with (
            tc.tile_pool(name="wconst", bufs=1) as wconst,
            tc.tile_pool(name="sstage", bufs=4) as sstage,
            tc.tile_pool(name="gpool", bufs=2) as gpool,
            tc.tile_pool(name="x8pool", bufs=N_CHUNKS) as x8pool,
            tc.tile_pool(name="xpool", bufs=5) as xpool,
            tc.tile_pool(name="spool", bufs=N_CHUNKS) as spool,
            tc.tile_pool(name="ppool", bufs=N_CHUNKS) as ppool,
            tc.tile_pool(name="psum", bufs=8, space="PSUM") as psum,
            tc.tile_pool(name="opool", bufs=3) as opool,
        ):
            # ---- PE warm-up: dummy MMs on a memset tile, no DMA deps ----
            wtile = wconst.tile([P, M_CHUNK], BF16, name="wtile")
            nc.vector.memset(wtile[:], 0.0)
            # Dummy silu primes the ACT table load off the critical path.
            sdum = wconst.tile([P, 4], BF16, name="sdum")
            nc.scalar.activation(sdum[:], wtile[:, 0:4], AF.Silu)
            warm_pt = psum.tile([P, M_CHUNK], F32, name="pt", tag="pt")
            for _ in range(N_WARM):
                nc.tensor.matmul(warm_pt[:], wtile[:, 0:P], wtile[:],
                                 start=True, stop=True)

            # ---- all inputs on the Sync ring, in IN_ORDER ----
            wb8 = wconst.tile([P, KB, 2, O_LOC], FP8, name="wb8")
            nc.sync.dma_start(wb8[:], bt8[:])
            x8s = [None] * N_CHUNKS
            xbs = [None] * N_CHUNKS
            sbs = [None] * N_CHUNKS
            stgs = [None] * (KT // 2)
            for kind, i in IN_ORDER:
                if kind == "x8":
                    x8 = x8pool.tile([P, KT, M_CHUNK], FP8, name="x8",
                                     tag="x8")
                    nc.sync.dma_start(x8[:], x8t[i])
                    x8s[i] = x8
                elif kind == "st":
                    stg = sstage.tile([P, 2, G, O_LOC], BF16, name="stg",
                                      tag="stg")
                    nc.sync.dma_start(stg[:], st[i])
                    stgs[i] = stg
                else:
                    xb = xpool.tile([P, KT, M_CHUNK], BF16, name="xb",
                                    tag="xb")
                    nc.sync.dma_start(xb[:], xt[i])
                    sb = spool.tile([P, KT, M_CHUNK], BF16, name="sb",
                                    tag="sb")
                    nc.scalar.activation(sb[:], xb[:], AF.Silu)
                    xbs[i] = xb
                    sbs[i] = sb

            # ---- spline g-sum on DVE, threaded into pass 1 below ----
            ws = wconst.tile([P, KT, O_LOC], BF16, name="ws")

            def gsum(k):
                stg = stgs[k // 2]
                k2 = k % 2
                a1 = gpool.tile([P, 4, O_LOC], BF16, name="a1", tag="a1")
                nc.vector.tensor_add(a1[:], stg[:, k2, 0:4], stg[:, k2, 4:8])
                a2 = gpool.tile([P, 2, O_LOC], BF16, name="a2", tag="a2")
                nc.vector.tensor_add(a2[:], a1[:, 0:2], a1[:, 2:4])
                nc.vector.tensor_add(ws[:, k], a2[:, 0], a2[:, 1])

            # ---- pass 1: fp8 DoubleRow base matmuls -> bf16 partials ----
            parts = []
            for ch in range(N_CHUNKS):
                part = ppool.tile([P, OT, M_CHUNK], BF16, name="part",
                                  tag="part")
                for o in range(OT):
                    pt = psum.tile([P, M_CHUNK], F32, name="pt", tag="pt")
                    osl = slice(P * o, P * (o + 1))
                    for kb in range(KB):
                        nc.tensor.matmul(
                            pt[:], wb8[:, kb, :, osl],
                            x8s[ch][:, 2 * kb:2 * kb + 2, :],
                            start=(kb == 0), stop=(kb == KB - 1),
                            perf_mode=DR,
                        )
                    nc.vector.tensor_copy(part[:, o], pt[:])
                parts.append(part)
                for k in GSUM_AFTER.get(ch, []):
                    gsum(k)

            # ---- pass 2: bf16 spline matmuls, fold partials at evict ----
            for ch in range(N_CHUNKS):
                otile = opool.tile([P, OT, M_CHUNK], BF16, name="otile",
                                   tag="otile")
                for o in range(OT):
                    pt = psum.tile([P, M_CHUNK], F32, name="pt", tag="pt")
                    osl = slice(P * o, P * (o + 1))
                    for k in range(KT):
                        nc.tensor.matmul(
                            pt[:], ws[:, k, osl], sbs[ch][:, k],
                            start=(k == 0), stop=(k == KT - 1),
                        )
                    nc.vector.tensor_add(otile[:, o], pt[:], parts[ch][:, o])
                nc.gpsimd.dma_start(out[ch], otile[:])
    nc.compile()
    return nc


def _get_compiled():
    global _compiled
    if _compiled is None:
        _compiled = _build_kernel()
    return _compiled


def _shard_inputs(x, base_weight, spline_weight):
    """Full f32 inputs -> 8 per-core in_maps (layout + dtype cast)."""
    x = np.asarray(x, dtype=np.float32)
    base_weight = np.asarray(base_weight, dtype=np.float32)
    spline_weight = np.asarray(spline_weight, dtype=np.float32)

    xt_T = np.ascontiguousarray(x.T)                        # [1024, 8192] f32
    btf = np.ascontiguousarray(base_weight.T)               # [1024 i, 1024 o]

    # Per row-group r: [ch, p, k, m] with one contiguous block per chunk
    xts, x8ts = [], []
    for r in range(R_SPLIT):
        xs = xt_T[:, B_LOC * r:B_LOC * (r + 1)]             # [1024, 4096]
        xs4 = (xs.reshape(KT, P, N_CHUNKS, M_CHUNK)
                 .transpose(2, 1, 0, 3))
        xts.append(np.ascontiguousarray(xs4.astype(NP_BF16)))
        x8ts.append(np.ascontiguousarray(xs4.astype(NP_FP8)))

    bts, sts = [], []
    for c in range(C_SPLIT):
        osl = slice(O_LOC * c, O_LOC * (c + 1))
        btc = (btf[:, osl]                         # [1024 i, 256 o]
               .reshape(KB, 2, P, O_LOC)           # [kb, k2, p, o]
               .transpose(2, 0, 1, 3)              # [p, kb, k2, o]
               .astype(NP_FP8))
        bts.append(np.ascontiguousarray(btc))
        stc = (spline_weight[osl]                  # [256 o, 1024 i, 8 g]
               .transpose(1, 2, 0)                 # [1024 i, 8 g, 256 o]
               .astype(NP_BF16)
               .reshape(KT // 2, 2, P, G, O_LOC)
               .transpose(0, 2, 1, 3, 4))          # [t, p, k2, g, o]
        sts.append(np.ascontiguousarray(stc))

    in_maps = []
    for core in range(N_CORES):
        r, c = divmod(core, C_SPLIT)
        in_maps.append({"xt": xts[r], "x8t": x8ts[r],
                        "bt8": bts[c], "st": sts[c]})
    return in_maps


def _gather_output(results):
    out = np.empty((8192, 1024), dtype=np.float32)
    for core in range(N_CORES):
        r, c = divmod(core, C_SPLIT)
        oc = results[core]["out"].astype(np.float32)  # [8 ch, 128 p, 2 ot, 512 m]
        oc = oc.transpose(0, 3, 2, 1).reshape(B_LOC, O_LOC)
        out[B_LOC * r:B_LOC * (r + 1), O_LOC * c:O_LOC * (c + 1)] = oc
    return out


def run(trace=False, **inputs):
    """Run on the 8 NeuronCores; returns (out, BassKernelResults)."""
    nc = _get_compiled()
    in_maps = _shard_inputs(**inputs)
    res = run_bass_kernel_spmd(
        nc, in_maps, core_ids=list(range(N_CORES)), trace=trace)
    return _gather_output(res.results), res


def kernel(**inputs) -> np.ndarray:
    out, _ = run(trace=False, **inputs)
    return out


# revision 21
# speedup vs baseline: 1.2285x; 1.0324x over previous
"""KAN layer on 8 Trainium2 NeuronCores (Bass/Tile).

Computes out = x @ base_weight.T + silu(x) @ spline_weight.sum(-1).T
for x:[8192,1024] f32, base_weight:[1024,1024] f32,
spline_weight:[1024,1024,8] f32 -> out:[8192,1024] f32.

Strategy (self-contained, hardcoded for these shapes):
  * 2D shard over the 8 cores: batch split R=2, out-features split C=4.
    Core (r, c) computes out[4096r:4096(r+1), 256c:256(c+1)].
  * Host prep is layout + dtype cast: x/spline ship bf16; the base
    path (x @ Wb^T) ships fp8e4 and runs DoubleRow matmuls (K=256 per
    MM) — it carries only ~16% of the output variance, so fp8 noise
    there is ~0.5% of the output. The spline path stays bf16.
  * Weight-stationary transposed formulation on device: psum[128o,
    512b] += W^T[128i, 128o].T @ x^T[128i, 512b].
  * Two decoupled passes over the batch: the base pass (fp8, fed by
    the small fp8 x stream that lands first) runs early and parks its
    PSUM tiles as bf16 partials in SBUF; the spline pass (bf16, fed
    by the later-landing spline weights + serial silu chain) runs
    second and folds the partials back in at PSUM-evict time.
  * ALL inputs ride ONE hand-ordered Sync-ring FIFO (SDMA round-robin
    is per-packet, so competing rings dilute each other's bandwidth —
    a single ring gets the full ~430 GB/s in exactly the order the
    compute needs). Outputs ride the otherwise idle GpSimd ring.
  * Spline g-axis reduce on Vector, positioned in the queue to match
    spline-weight arrival; silu on Scalar (primed by a dummy op so
    its table load is off the critical path); PE warm-up MMs bring
    the HAM clock to 2.4 GHz before real work.
  * Output is written bf16 and upcast to f32 on gather.
    End-to-end rel err vs the f32 reference ~8e-3.
"""
import sys

for _p in ("/opt/trn_rl_repo",):
    if _p not in sys.path:
        sys.path.insert(0, _p)

import ml_dtypes
import numpy as np

import concourse.bass as bass  # noqa: F401  (bass must import before mybir use)
import concourse.mybir as mybir
import concourse.tile as tile
from concourse import bacc
from concourse.bass_utils import run_bass_kernel_spmd

P = 128
IN_F = 1024
G = 8
N_CORES = 8
R_SPLIT = 2
C_SPLIT = 4
B_LOC = 8192 // R_SPLIT      # 4096 batch rows per core
O_LOC = 1024 // C_SPLIT      # 256 out features per core
KT = IN_F // P               # 8 k-tiles over in_features
KB = KT // 2                 # 4 DoubleRow k-blocks of 256
M_CHUNK = 512
N_CHUNKS = B_LOC // M_CHUNK  # 8
OT = O_LOC // P              # 2 out-feature tiles of 128
N_WARM = 8                   # dummy MMs to warm the PE HAM clock

F32 = mybir.dt.float32
BF16 = mybir.dt.bfloat16
FP8 = mybir.dt.float8e4
AF = mybir.ActivationFunctionType
DR = mybir.MatmulPerfMode.DoubleRow
NP_BF16 = ml_dtypes.bfloat16
NP_FP8 = ml_dtypes.float8_e4m3

# Single-ring input order (by need time): fp8 x quads fuel the early
# base pass, the first bf16 chunks keep the silu chain fed, spline
# weights land in time for the spline pass, remaining bf16 x trails.
IN_ORDER = [("x8", 0), ("x8", 1), ("xb", 0), ("x8", 2), ("x8", 3),
            ("xb", 1), ("st", 0), ("st", 1), ("st", 2), ("st", 3),
            ("xb", 2), ("xb", 3), ("xb", 4), ("xb", 5), ("xb", 6),
            ("xb", 7)]

# g-sum k-tiles threaded into the DVE queue after these base-pass
# chunks' evicts (matched to spline-weight arrival times).
GSUM_AFTER = {4: [0, 1], 5: [2, 3], 6: [4, 5], 7: [6, 7]}

_compiled = None


def _build_kernel():
    nc = bacc.Bacc(None, target_bir_lowering=False, num_devices=N_CORES)
    # x^T tiles: [ch, p, k, m] = x[r*4096 + ch*512 + m, k*128 + p]
    xt = nc.dram_tensor("xt", [N_CHUNKS, P, KT, M_CHUNK], BF16,
                        kind="ExternalInput")
    # fp8 x packed as 4 pairs of 2 chunks -> 8KB/partition rows
    x8t = nc.dram_tensor("x8t", [4, P, 2, KT, M_CHUNK], FP8,
                         kind="ExternalInput")
    # W_base^T fp8 DoubleRow layout: bt8[p, kb, k2, o] =
    #   base_weight[c*256 + o, (2kb+k2)*128 + p]
    bt8 = nc.dram_tensor("bt8", [P, KB, 2, O_LOC], FP8, kind="ExternalInput")
    # spline^T: st[t, p, k2, g, o] = spline_weight[c*256+o, (2t+k2)*128+p, g]
    st = nc.dram_tensor("st", [KT // 2, P, 2, G, O_LOC], BF16,
                        kind="ExternalInput")
    # out^T tiles: out[ch, p, ot, m] = result[r*4096+ch*512+m, c*256+ot*128+p]
    out = nc.dram_tensor("out", [N_CHUNKS, P, OT, M_CHUNK], BF16,
                         kind="ExternalOutput")

    with tile.TileContext(nc) as tc:
        with (
            tc.tile_pool(name="wconst", bufs=1) as wconst,
            tc.tile_pool(name="sstage", bufs=4) as sstage,
            tc.tile_pool(name="gpool", bufs=2) as gpool,
            tc.tile_pool(name="x8pool", bufs=4) as x8pool,
            tc.tile_pool(name="xpool", bufs=5) as xpool,
            tc.tile_pool(name="spool", bufs=N_CHUNKS) as spool,
            tc.tile_pool(name="ppool", bufs=N_CHUNKS) as ppool,
            tc.tile_pool(name="psum", bufs=8, space="PSUM") as psum,
            tc.tile_pool(name="opool", bufs=3) as opool,
        ):
            # ---- PE warm-up: dummy MMs on a memset tile, no DMA deps ----
            wtile = wconst.tile([P, M_CHUNK], BF16, name="wtile")
            nc.vector.memset(wtile[:], 0.0)
            # Dummy silu primes the ACT table load off the critical path.
            sdum = wconst.tile([P, 4], BF16, name="sdum")
            nc.scalar.activation(sdum[:], wtile[:, 0:4], AF.Silu)
            warm_pt = psum.tile([P, M_CHUNK], F32, name="pt", tag="pt")
            for _ in range(N_WARM):
                nc.tensor.matmul(warm_pt[:], wtile[:, 0:P], wtile[:],
                                 start=True, stop=True)

            # ---- all inputs on the Sync ring, in IN_ORDER ----
            wb8 = wconst.tile([P, KB, 2, O_LOC], FP8, name="wb8")
            nc.sync.dma_start(wb8[:], bt8[:])
            x8qs = [None] * 4
            xbs = [None] * N_CHUNKS
            sbs = [None] * N_CHUNKS
            stgs = [None] * (KT // 2)
            for kind, i in IN_ORDER:
                if kind == "x8":
                    x8 = x8pool.tile([P, 2, KT, M_CHUNK], FP8, name="x8",
                                     tag="x8")
                    nc.sync.dma_start(x8[:], x8t[i])
                    x8qs[i] = x8
                elif kind == "st":
                    stg = sstage.tile([P, 2, G, O_LOC], BF16, name="stg",
                                      tag="stg")
                    nc.sync.dma_start(stg[:], st[i])
                    stgs[i] = stg
                else:
                    xb = xpool.tile([P, KT, M_CHUNK], BF16, name="xb",
                                    tag="xb")
                    nc.sync.dma_start(xb[:], xt[i])
                    sb = spool.tile([P, KT, M_CHUNK], BF16, name="sb",
                                    tag="sb")
                    nc.scalar.activation(sb[:], xb[:], AF.Silu)
                    xbs[i] = xb
                    sbs[i] = sb

            # ---- spline g-sum on DVE, threaded into pass 1 below ----
            ws = wconst.tile([P, KT, O_LOC], BF16, name="ws")

            def gsum(k):
                stg = stgs[k // 2]
                k2 = k % 2
                a1 = gpool.tile([P, 4, O_LOC], BF16, name="a1", tag="a1")
                nc.vector.tensor_add(a1[:], stg[:, k2, 0:4], stg[:, k2, 4:8])
                a2 = gpool.tile([P, 2, O_LOC], BF16, name="a2", tag="a2")
                nc.vector.tensor_add(a2[:], a1[:, 0:2], a1[:, 2:4])
                nc.vector.tensor_add(ws[:, k], a2[:, 0], a2[:, 1])

            # ---- pass 1: fp8 DoubleRow base matmuls -> bf16 partials ----
            parts = []
            for ch in range(N_CHUNKS):
                part = ppool.tile([P, OT, M_CHUNK], BF16, name="part",
                                  tag="part")
                for o in range(OT):
                    pt = psum.tile([P, M_CHUNK], F32, name="pt", tag="pt")
                    osl = slice(P * o, P * (o + 1))
                    for kb in range(KB):
                        nc.tensor.matmul(
                            pt[:], wb8[:, kb, :, osl],
                            x8qs[ch // 2][:, ch % 2, 2 * kb:2 * kb + 2, :],
                            start=(kb == 0), stop=(kb == KB - 1),
                            perf_mode=DR,
                        )
                    nc.vector.tensor_copy(part[:, o], pt[:])
                parts.append(part)
                for k in GSUM_AFTER.get(ch, []):
                    gsum(k)

            # ---- pass 2: bf16 spline matmuls, fold partials at evict ----
            for ch in range(N_CHUNKS):
                otile = opool.tile([P, OT, M_CHUNK], BF16, name="otile",
                                   tag="otile")
                for o in range(OT):
                    pt = psum.tile([P, M_CHUNK], F32, name="pt", tag="pt")
                    osl = slice(P * o, P * (o + 1))
                    for k in range(KT):
                        nc.tensor.matmul(
                            pt[:], ws[:, k, osl], sbs[ch][:, k],
                            start=(k == 0), stop=(k == KT - 1),
                        )
                    nc.vector.tensor_add(otile[:, o], pt[:], parts[ch][:, o])
                    if ch == N_CHUNKS - 1:
                        # split the last write so o=0 overlaps o=1's MMs
                        nc.gpsimd.dma_start(out[ch, :, o], otile[:, o])
                if ch < N_CHUNKS - 1:
                    nc.gpsimd.dma_start(out[ch], otile[:])
    nc.compile()
    return nc


def _get_compiled():
    global _compiled
    if _compiled is None:
        _compiled = _build_kernel()
    return _compiled


def _shard_inputs(x, base_weight, spline_weight):
    """Full f32 inputs -> 8 per-core in_maps (layout + dtype cast)."""
    x = np.asarray(x, dtype=np.float32)
    base_weight = np.asarray(base_weight, dtype=np.float32)
    spline_weight = np.asarray(spline_weight, dtype=np.float32)

    xt_T = np.ascontiguousarray(x.T)                        # [1024, 8192] f32
    btf = np.ascontiguousarray(base_weight.T)               # [1024 i, 1024 o]

    # Per row-group r: [ch, p, k, m] with one contiguous block per chunk
    xts, x8ts = [], []
    for r in range(R_SPLIT):
        xs = xt_T[:, B_LOC * r:B_LOC * (r + 1)]             # [1024, 4096]
        xs4 = (xs.reshape(KT, P, N_CHUNKS, M_CHUNK)
                 .transpose(2, 1, 0, 3))                    # [ch, p, k, m]
        xts.append(np.ascontiguousarray(xs4.astype(NP_BF16)))
        x8q = (xs4.astype(NP_FP8)
               .reshape(4, 2, P, KT, M_CHUNK)
               .transpose(0, 2, 1, 3, 4))                   # [q, p, c2, k, m]
        x8ts.append(np.ascontiguousarray(x8q))

    bts, sts = [], []
    for c in range(C_SPLIT):
        osl = slice(O_LOC * c, O_LOC * (c + 1))
        btc = (btf[:, osl]                         # [1024 i, 256 o]
               .reshape(KB, 2, P, O_LOC)           # [kb, k2, p, o]
               .transpose(2, 0, 1, 3)              # [p, kb, k2, o]
               .astype(NP_FP8))
        bts.append(np.ascontiguousarray(btc))
        stc = (spline_weight[osl]                  # [256 o, 1024 i, 8 g]
               .transpose(1, 2, 0)                 # [1024 i, 8 g, 256 o]
               .astype(NP_BF16)
               .reshape(KT // 2, 2, P, G, O_LOC)
               .transpose(0, 2, 1, 3, 4))          # [t, p, k2, g, o]
        sts.append(np.ascontiguousarray(stc))

    in_maps = []
    for core in range(N_CORES):
        r, c = divmod(core, C_SPLIT)
        in_maps.append({"xt": xts[r], "x8t": x8ts[r],
                        "bt8": bts[c], "st": sts[c]})
    return in_maps


def _gather_output(results):
    out = np.empty((8192, 1024), dtype=np.float32)
    for core in range(N_CORES):
        r, c = divmod(core, C_SPLIT)
        oc = results[core]["out"].astype(np.float32)  # [8 ch, 128 p, 2 ot, 512 m]
        oc = oc.transpose(0, 3, 2, 1).reshape(B_LOC, O_LOC)
        out[B_LOC * r:B_LOC * (r + 1), O_LOC * c:O_LOC * (c + 1)] = oc
    return out


def run(trace=False, **inputs):
    """Run on the 8 NeuronCores; returns (out, BassKernelResults)."""
    nc = _get_compiled()
    in_maps = _shard_inputs(**inputs)
    res = run_bass_kernel_spmd(
        nc, in_maps, core_ids=list(range(N_CORES)), trace=trace)
    return _gather_output(res.results), res


def kernel(**inputs) -> np.ndarray:
    out, _ = run(trace=False, **inputs)
    return out
